# revision 11
# baseline (speedup 1.0000x reference)
"""DeepFM (embedding gather + FM + 5-layer seq-1 attention + head) on 8 trn2 cores.

Strategy: data-parallel over batch (2048 rows/core). Inside each core:
  - fused emb1+emb2 table (F*V, 17) f32; ONE batched indirect-DMA gather per
    128-row tile ([128,39] offset AP -> 39*17 f32 per row) into batch-major G
  - Xv scaling, L2-norm over fields, FM second-order stats via strided DVE/ACT
  - xbar DMA-transposes (bf16) to feature-major; attention stack as bf16
    matmuls with all weights resident in SBUF (loaded once, per-layer tiles)
  - Wo_l folded into Wv_{l+1} on host (v' = att @ (Wo Wv)) -> 3 projections
    per layer instead of 4; final Wo_4 folded into the m3 head matrix
  - M-blocks zero-padded to 128 (640-wide) so FWL stays enabled
  - all linear biases folded in via augmented ones-rows / bias rows
"""
import numpy as np
import ml_dtypes

import concourse.bass as bass
import concourse.mybir as mybir
from concourse.tile import TileContext
from concourse.vector_clock import ScopedClock
from concourse.bass_utils import run_bass_kernel_spmd

F32 = mybir.dt.float32
F32R = mybir.dt.float32r
BF16 = mybir.dt.bfloat16
I32 = mybir.dt.int32
AF = mybir.ActivationFunctionType
OP = mybir.AluOpType

# problem constants (hardcoded per contract)
N = 16384
F = 39
V = 100000
E = 16
EW = E + 1          # fused [emb1 | emb2] row width
D = F * E           # 624
L = 5
NCORES = 8
NPC = N // NCORES   # 2048 rows per core
NT = NPC // 128     # 16 tiles of 128 rows
NB = 512            # matmul batch-chunk (free dim)
NCH = NPC // NB     # 4 chunks
TPC = NB // 128     # 4 tiles per chunk
KDIMS = [128, 128, 128, 128, 112]      # K-tiles over 624
KDIMS_AUG = [128, 128, 128, 128, 113]  # incl. ones/bias row at 112 of tail tile
WPL = 3 * 5 * 640   # weight cols per layer tile (3 proj x 5 kb x 640 m-cols)

MAX_WAITS = 1

LAST_RESULT = None  # test harness reads exec_time_ns from here


class SplitWaitTileContext(TileContext):
    """Walrus (CoreV3) accepts at most one sync-wait command per instruction;
    Tile can emit several. Split extras onto preceding same-engine NOPs, and
    do the same for the kernel-tail drain."""

    def _add_instruction(self, inst):
        si = inst.sync_info
        if si is not None and len(si.on_wait) > MAX_WAITS:
            waits = list(si.on_wait)
            head, tail = waits[:-MAX_WAITS], waits[-MAX_WAITS:]
            for i in range(0, len(head), MAX_WAITS):
                nop = mybir.InstNoOp(
                    name=self.nc.get_next_instruction_name(),
                    sync_info=mybir.SyncInfo(
                        on_wait=head[i : i + MAX_WAITS], on_update=[]
                    ),
                    bass_nofuse=True,
                    engine=inst.engine,
                )
                super()._add_instruction(nop)
            inst.sync_info = mybir.SyncInfo(on_wait=tail, on_update=si.on_update)
        super()._add_instruction(inst)

    def _drain_and_barrier(self, tick_clock, wait_clock):
        nc = self.nc
        probe = nc.sync.nop(nofuse=True, hint="tail_wait_probe")
        wait_clock.add_sem_waits(
            probe.ins, ScopedClock({None: tick_clock.global_clock})
        )
        waits = list(probe.ins.sync_info.on_wait)
        probe.ins.sync_info.on_wait = waits[:MAX_WAITS]
        for i in range(MAX_WAITS, len(waits), MAX_WAITS):
            nop = nc.sync.nop(nofuse=True, hint="tail_wait_split")
            nop.ins.sync_info = mybir.SyncInfo(
                on_wait=waits[i : i + MAX_WAITS], on_update=[]
            )
        drain_inst = nc.sync.drain()
        wait_clock.add_sem_waits(
            drain_inst.ins, ScopedClock({None: tick_clock.global_clock})
        )
        if len(drain_inst.ins.sync_info.on_wait) > MAX_WAITS:
            drain_inst.ins.sync_info.on_wait = []
        nc.all_engine_barrier()
        assert self.sems is not None
        popped = nc._tile_sem_poison_stack.pop()
        assert popped is self._sem_poison
        nc.clear_and_free_semaphores(list(self.sems.allocated().values()))
        nc.all_engine_barrier()


def round_f32r(a):
    """Round-to-nearest-even fp32 -> e8m11 (low 12 mantissa bits zero)."""
    a = np.ascontiguousarray(a, dtype=np.float32)
    u = a.view(np.uint32)
    r = (u + np.uint32(0x7FF) + ((u >> np.uint32(12)) & np.uint32(1))) & np.uint32(
        0xFFFFF000
    )
    return r.view(np.float32)


def to_bf16(a):
    return np.ascontiguousarray(np.asarray(a, dtype=np.float32)).astype(
        ml_dtypes.bfloat16
    )


def build_nc():
    nc = bass.Bass()

    tab = nc.declare_dram_parameter("tab", [F * V, EW], F32, isOutput=False)
    idx = nc.declare_dram_parameter("idx", [128, NT * F], I32, isOutput=False)
    xv = nc.declare_dram_parameter("xv", [128, NT * F], F32, isOutput=False)
    wall = nc.declare_dram_parameter("wall", [L, 128, WPL], BF16, isOutput=False)
    am = nc.declare_dram_parameter("am", [128, 5 * F], BF16, isOutput=False)
    bm = nc.declare_dram_parameter("bm", [F, 5 * 128], BF16, isOutput=False)
    m12 = nc.declare_dram_parameter("m12", [56, 9], BF16, isOutput=False)
    m3 = nc.declare_dram_parameter("m3", [128, 5 * 4], BF16, isOutput=False)
    ffw9 = nc.declare_dram_parameter("ffw9", [9, 13], BF16, isOutput=False)
    ffw3 = nc.declare_dram_parameter("ffw3", [4, 13], BF16, isOutput=False)
    fdw = nc.declare_dram_parameter("fdw", [13, 2], BF16, isOutput=False)
    y = nc.declare_dram_parameter("y", [NPC, 2], F32, isOutput=True)

    with SplitWaitTileContext(nc) as tc:
        with (
            tc.tile_pool(name="const", bufs=1) as cp,
            tc.tile_pool(name="persist", bufs=1) as pp,
            tc.tile_pool(name="gpool", bufs=3) as gp,
            tc.tile_pool(name="work", bufs=2) as wk_p,
            tc.tile_pool(name="small", bufs=2) as sp,
            tc.tile_pool(name="attp", bufs=1) as ap_p,
            # 8 PSUM banks total: q1 + k2 + v2 + s1 + b1 + h1
            tc.tile_pool(name="ps", bufs=1, space="PSUM") as psp,
        ):
            # ---- constants / persistent tiles ----
            # sync-ring DMAs: idx/xv first (gathers + scaling need them early),
            # then the phase-0 transposes share this ring.
            idx_sb = cp.tile([128, NT * F], I32)
            nc.sync.dma_start(out=idx_sb[:], in_=idx[:])
            xv_sb = cp.tile([128, NT * F], F32)
            nc.sync.dma_start(out=xv_sb[:], in_=xv[:])
            # scalar-ring DMAs: weights (layer-granular so layer 0 lands fast)
            wl_sb = []
            for l in range(L):
                w = cp.tile([128, WPL], BF16, name=f"wl{l}")
                nc.scalar.dma_start(out=w[:], in_=wall[l, :, :])
                wl_sb.append(w)
            a_sb = cp.tile([128, 5 * F], BF16)
            nc.scalar.dma_start(out=a_sb[:], in_=am[:])
            b_sb = cp.tile([F, 5 * 128], BF16)
            nc.scalar.dma_start(out=b_sb[:], in_=bm[:])
            m12_sb = cp.tile([56, 9], BF16)
            nc.scalar.dma_start(out=m12_sb[:], in_=m12[:])
            m3_sb = cp.tile([128, 5 * 4], BF16)
            nc.scalar.dma_start(out=m3_sb[:], in_=m3[:])
            ffw9_sb = cp.tile([9, 13], BF16)
            nc.scalar.dma_start(out=ffw9_sb[:], in_=ffw9[:])
            ffw3_sb = cp.tile([4, 13], BF16)
            nc.scalar.dma_start(out=ffw3_sb[:], in_=ffw3[:])
            fdw_sb = cp.tile([13, 2], BF16)
            nc.scalar.dma_start(out=fdw_sb[:], in_=fdw[:])

            XVT = [
                pp.tile([128, NPC], BF16, tag=f"xvt{i}", name=f"xvt{i}")
                for i in range(5)
            ]
            headT = pp.tile([128, NPC], BF16, tag="headT")
            out_sb = pp.tile([128, NT * 2], F32, tag="outsb")

            # ---- phase 0: gather + FM + normalize + transpose, per 128-row tile
            for t in range(NT):
                g = gp.tile([128, F * EW], F32, tag="g")
                g3 = g[:].rearrange("p (f j) -> p f j", j=EW)
                # one indirect DMA per field: multi-column offset APs are not
                # supported by the SWDGE path (reads garbage offsets)
                for f in range(F):
                    col = t * F + f
                    nc.gpsimd.indirect_dma_start(
                        out=g[:, f * EW : (f + 1) * EW],
                        out_offset=None,
                        in_=tab[:],
                        in_offset=bass.IndirectOffsetOnAxis(
                            ap=idx_sb[:, col : col + 1], axis=0
                        ),
                    )
                # scale by Xv (also scales the emb1 slot -> f1w)
                nc.vector.tensor_tensor(
                    out=g3,
                    in0=g3,
                    in1=xv_sb[:, t * F : (t + 1) * F]
                    .unsqueeze(2)
                    .to_broadcast([128, F, EW]),
                    op=OP.mult,
                )
                g_xv = g3[:, :, 1:]                      # (p, f, e)
                g_ef = g_xv.transpose([0, 2, 1])         # (p, e, f) view
                sq = wk_p.tile([128, D], F32, tag="sq")
                sq_v = sq[:].rearrange("p (e f) -> p e f", f=F)
                nc.scalar.activation(out=sq_v, in_=g_ef, func=AF.Square)
                ss = sp.tile([128, 16], F32, tag="ss")
                nc.vector.reduce_sum(out=ss[:], in_=sq_v, axis=mybir.AxisListType.X)
                tt = sp.tile([128, 16], F32, tag="tt")
                nc.vector.reduce_sum(out=tt[:], in_=g_ef, axis=mybir.AxisListType.X)
                mx = sp.tile([128, 16], F32, tag="mx")
                nc.vector.tensor_scalar_max(out=mx[:], in0=ss[:], scalar1=1e-24)
                rt = sp.tile([128, 16], F32, tag="rt")
                nc.scalar.sqrt(out=rt[:], in_=mx[:])
                inv = sp.tile([128, 16], F32, tag="inv")
                nc.vector.reciprocal(out=inv[:], in_=rt[:])
                # normalized xv in bf16, contiguous (f, e) layout + ones col
                xvt_bf = wk_p.tile([128, 5 * 128], BF16, tag="xvtbf")
                nc.vector.memset(xvt_bf[:, D : D + 1], 1.0)
                xv_v = xvt_bf[:, :D].rearrange("p (f e) -> p f e", e=E)
                nc.vector.tensor_tensor(
                    out=xv_v,
                    in0=g_xv,
                    in1=inv[:].unsqueeze(1).to_broadcast([128, F, E]),
                    op=OP.mult,
                )
                # head tile: [f1w | f2 | 1] (cols 56.. are never consumed)
                head_bf = wk_p.tile([128, 128], BF16, tag="headbf")
                nc.vector.memset(head_bf[:, 55:56], 1.0)
                nc.scalar.activation(out=head_bf[:, 0:F], in_=g3[:, :, 0], func=AF.Copy)
                u = sp.tile([128, 16], F32, tag="u")
                nc.vector.tensor_tensor(out=u[:], in0=tt[:], in1=inv[:], op=OP.mult)
                u2 = sp.tile([128, 16], F32, tag="u2")
                nc.vector.tensor_tensor(out=u2[:], in0=u[:], in1=u[:], op=OP.mult)
                w1 = sp.tile([128, 16], F32, tag="w1")
                nc.vector.tensor_tensor(out=w1[:], in0=ss[:], in1=inv[:], op=OP.mult)
                w2 = sp.tile([128, 16], F32, tag="w2")
                nc.vector.tensor_tensor(out=w2[:], in0=w1[:], in1=inv[:], op=OP.mult)
                dd = sp.tile([128, 16], F32, tag="dd")
                nc.vector.tensor_tensor(
                    out=dd[:], in0=u2[:], in1=w2[:], op=OP.subtract
                )
                nc.vector.tensor_scalar_mul(
                    out=head_bf[:, F:55], in0=dd[:], scalar1=0.5
                )
                # xbar transposes to feature-major (bf16, 128x128 blocks)
                for cc in range(5):
                    nc.sync.dma_start_transpose(
                        out=XVT[cc][:, t * 128 : (t + 1) * 128],
                        in_=xvt_bf[:, cc * 128 : (cc + 1) * 128],
                    )
                nc.sync.dma_start_transpose(
                    out=headT[:, t * 128 : (t + 1) * 128], in_=head_bf[:]
                )

            # ---- attention: chunk-major, weights resident, Wo folded forward
            att = [
                [
                    ap_p.tile([128, NB], BF16, tag=f"att{par}_{kb}", name=f"att{par}_{kb}")
                    for kb in range(5)
                ]
                for par in range(2)
            ]
            # bias path: ones row at K-row 112 of tail block (both parities);
            # the att DVE writes only touch rows [:112] of block 4, so row 112
            # survives all chunks/layers. Partition offset must be 32-aligned:
            # memset [96:128] — rows 96..111 are rewritten by every att write,
            # rows 113..127 are never read.
            nc.vector.memset(att[0][4][96:128, :], 1.0)
            nc.vector.memset(att[1][4][96:128, :], 1.0)
            for c in range(NCH):
                cs = slice(c * NB, (c + 1) * NB)
                for l in range(L):
                    wl = wl_sb[l]

                    def wslice(proj, kb, m):
                        base = proj * 3200 + kb * 640 + m * 128
                        return wl[:, base : base + 128]

                    Xsrc = XVT if l == 0 else att[(l - 1) % 2]
                    dst = att[l % 2]
                    # -- scores phase: psq/psk per m, p = q*k, pss += am.T @ p
                    pss = psp.tile([F, NB], F32, tag="s", bufs=1)
                    p_tiles = [None] * 5
                    for m in range(5):
                        psq = psp.tile([128, NB], F32, tag="q", bufs=1)
                        for kb in range(5):
                            kw = KDIMS_AUG[kb]
                            nc.tensor.matmul(
                                out=psq[:, :],
                                lhsT=wslice(0, kb, m)[:kw, :],
                                rhs=XVT[kb][:kw, cs],
                                start=(kb == 0),
                                stop=(kb == 4),
                            )
                        q_sb = wk_p.tile([128, NB], BF16, tag="qsb")
                        nc.scalar.activation(
                            out=q_sb[:, :], in_=psq[:, :], func=AF.Copy
                        )
                        psk = psp.tile([128, NB], F32, tag="k", bufs=2)
                        for kb in range(5):
                            kw = KDIMS_AUG[kb]
                            nc.tensor.matmul(
                                out=psk[:, :],
                                lhsT=wslice(1, kb, m)[:kw, :],
                                rhs=XVT[kb][:kw, cs],
                                start=(kb == 0),
                                stop=(kb == 4),
                            )
                        p_sb = wk_p.tile([128, NB], BF16, tag="psb")
                        nc.vector.tensor_tensor(
                            out=p_sb[:, :],
                            in0=q_sb[:, :],
                            in1=psk[:, :],
                            op=OP.mult,
                        )
                        p_tiles[m] = p_sb
                        # lag the am matmul one m-group so PE doesn't stall on
                        # the ACT/DVE round trip
                        if m >= 1:
                            nc.tensor.matmul(
                                out=pss[:, :],
                                lhsT=a_sb[:, (m - 1) * F : m * F],
                                rhs=p_tiles[m - 1][:, :],
                                start=(m - 1 == 0),
                                stop=False,
                            )
                    nc.tensor.matmul(
                        out=pss[:, :],
                        lhsT=a_sb[:, 4 * F : 5 * F],
                        rhs=p_tiles[4][:, :],
                        start=False,
                        stop=True,
                    )
                    s_sb = wk_p.tile([F, NB], BF16, tag="ssb")
                    nc.vector.tensor_copy(out=s_sb[:], in_=pss[:])
                    # -- value phase: psv per m -> SBUF copy, psb broadcast,
                    # att = v*s (v in SBUF: PSUM has one DVE read port)
                    v_tiles = [None] * 5
                    for m in range(5):
                        psv = psp.tile([128, NB], F32, tag="v", bufs=2)
                        for kb in range(5):
                            kw = KDIMS_AUG[kb]
                            nc.tensor.matmul(
                                out=psv[:, :],
                                lhsT=wslice(2, kb, m)[:kw, :],
                                rhs=Xsrc[kb][:kw, :] if l > 0 else XVT[kb][:kw, cs],
                                start=(kb == 0),
                                stop=(kb == 4),
                            )
                        v_sb = wk_p.tile([128, NB], BF16, tag="vsb")
                        nc.scalar.activation(
                            out=v_sb[:, :], in_=psv[:, :], func=AF.Copy
                        )
                        v_tiles[m] = v_sb
                        if m >= 1:
                            mm = m - 1
                            psb = psp.tile([128, NB], F32, tag="b", bufs=1)
                            nc.tensor.matmul(
                                out=psb[:, :],
                                lhsT=b_sb[:, mm * 128 : (mm + 1) * 128],
                                rhs=s_sb[:, :],
                                start=True,
                                stop=True,
                            )
                            ow = 112 if mm == 4 else 128
                            nc.vector.tensor_tensor(
                                out=dst[mm][:ow, :],
                                in0=v_tiles[mm][:ow, :],
                                in1=psb[:ow, :],
                                op=OP.mult,
                            )
                    psb = psp.tile([128, NB], F32, tag="b", bufs=1)
                    nc.tensor.matmul(
                        out=psb[:, :],
                        lhsT=b_sb[:, 4 * 128 : 5 * 128],
                        rhs=s_sb[:, :],
                        start=True,
                        stop=True,
                    )
                    nc.vector.tensor_tensor(
                        out=dst[4][:112, :],
                        in0=v_tiles[4][:112, :],
                        in1=psb[:112, :],
                        op=OP.mult,
                    )

                # ---- head (att parity 0 holds layer-5 output) ----
                fin = att[(L - 1) % 2]
                ffin12 = wk_p.tile([9, NB], BF16, tag="ffin12")
                ffin3 = wk_p.tile([4, NB], BF16, tag="ffin3")
                ffout = wk_p.tile([13, NB], BF16, tag="ffout")
                ph12 = psp.tile([9, NB], F32, tag="h", bufs=1)
                nc.tensor.matmul(
                    out=ph12[:, :],
                    lhsT=m12_sb[:],
                    rhs=headT[:56, cs],
                    start=True,
                    stop=True,
                )
                nc.vector.tensor_copy(out=ffin12[:], in_=ph12[:, :])
                ph3 = psp.tile([4, NB], F32, tag="h", bufs=1)
                for kb in range(5):
                    kw = KDIMS_AUG[kb]
                    nc.tensor.matmul(
                        out=ph3[:, :],
                        lhsT=m3_sb[:kw, kb * 4 : (kb + 1) * 4],
                        rhs=fin[kb][:kw, :],
                        start=(kb == 0),
                        stop=(kb == 4),
                    )
                nc.vector.tensor_copy(out=ffin3[:], in_=ph3[:, :])
                pf = psp.tile([13, NB], F32, tag="h", bufs=1)
                nc.tensor.matmul(
                    out=pf[:, :], lhsT=ffw9_sb[:], rhs=ffin12[:],
                    start=True, stop=False,
                )
                nc.tensor.matmul(
                    out=pf[:, :], lhsT=ffw3_sb[:], rhs=ffin3[:],
                    start=False, stop=True,
                )
                nc.vector.tensor_scalar_max(out=ffout[:], in0=pf[:, :], scalar1=0.0)
                for q in range(TPC):
                    cc = c * TPC + q
                    ptot = psp.tile([128, 2], F32, tag="h", bufs=1)
                    nc.tensor.matmul(
                        out=ptot[:, :],
                        lhsT=ffout[:, q * 128 : (q + 1) * 128],
                        rhs=fdw_sb[:],
                        start=True,
                        stop=True,
                    )
                    nc.vector.tensor_copy(
                        out=out_sb[:, cc * 2 : (cc + 1) * 2], in_=ptot[:, :]
                    )

            # final store: out_sb (128, NT, 2) -> y (NT*128, 2)
            nc.sync.dma_start(
                out=y[:].rearrange("(t p) j -> p t j", p=128),
                in_=out_sb[:].rearrange("p (t j) -> p t j", j=2),
            )

    return nc


def host_pack(Xi, Xv, emb1, emb2, Wq, bq, Wk, bk, Wv, bv, Wo, bo,
              m1_w, m1_b, m2_w, m2_b, m3_w, m3_b, ffw_w, ffw_b, fd_w, fd_b):
    """Preprocess full inputs into per-core input maps."""
    idxg = (
        np.arange(F, dtype=np.int64)[None, :] * V + np.asarray(Xi)[:, :, 0]
    ).astype(np.int32)                                    # (N, F)
    Xv = np.asarray(Xv, dtype=np.float32)
    tab = np.concatenate(
        [np.asarray(emb1).reshape(F * V, 1), np.asarray(emb2).reshape(F * V, E)],
        axis=1,
    ).astype(np.float32)                                  # (F*V, 17)

    # fold Wo into the next layer's Wv (and into the m3 head matrix)
    Wq = np.asarray(Wq, dtype=np.float64)
    Wk = np.asarray(Wk, dtype=np.float64)
    Wv = np.asarray(Wv, dtype=np.float64)
    Wo = np.asarray(Wo, dtype=np.float64)
    bq = np.asarray(bq, dtype=np.float64)
    bk = np.asarray(bk, dtype=np.float64)
    bv = np.asarray(bv, dtype=np.float64)
    bo = np.asarray(bo, dtype=np.float64)
    Wvp = np.empty_like(Wv)
    bvp = np.empty_like(bv)
    Wvp[0] = Wv[0]
    bvp[0] = bv[0]
    for l in range(1, L):
        Wvp[l] = Wo[l - 1] @ Wv[l]
        bvp[l] = bo[l - 1] @ Wv[l] + bv[l]
    m3wp = Wo[L - 1] @ np.asarray(m3_w, dtype=np.float64)
    m3bp = bo[L - 1] @ np.asarray(m3_w, dtype=np.float64) + np.asarray(
        m3_b, dtype=np.float64
    )

    # weights as lhsT tiles: [L, 128 K-rows, proj, kb, 640 m-cols] bf16,
    # bias at K-row 112 of kb=4; m-cols 624..639 zero padding (FWL needs 128)
    wall_h = np.zeros((L, 128, 3, 5, 640), dtype=np.float32)
    for proj, (W, b) in enumerate(((Wq, bq), (Wk, bk), (Wvp, bvp))):
        for kb in range(5):
            kw = KDIMS[kb]
            wall_h[:, :kw, proj, kb, :D] = W[:, kb * 128 : kb * 128 + kw, :]
        wall_h[:, 112, proj, 4, :D] = b
    wall_h = to_bf16(wall_h.reshape(L, 128, WPL))

    am_h = np.zeros((128, 5, F), dtype=np.float32)
    for m in range(5):
        for p in range(KDIMS[m]):
            d = m * 128 + p
            am_h[p, m, d // 16] = 0.25
    am_h = to_bf16(am_h.reshape(128, 5 * F))

    bm_h = np.zeros((F, 5, 128), dtype=np.float32)
    for m in range(5):
        for p in range(KDIMS[m]):
            d = m * 128 + p
            bm_h[d // 16, m, p] = 1.0
    bm_h = to_bf16(bm_h.reshape(F, 5 * 128))

    m12_h = np.zeros((56, 9), dtype=np.float32)
    m12_h[:F, 0:4] = np.asarray(m1_w, dtype=np.float32)
    m12_h[F:55, 4:8] = np.asarray(m2_w, dtype=np.float32)
    m12_h[55, 0:4] = np.asarray(m1_b, dtype=np.float32)
    m12_h[55, 4:8] = np.asarray(m2_b, dtype=np.float32)
    m12_h[55, 8] = 1.0   # ones-row producer (headT row 55 is all-ones)
    m12_h = to_bf16(m12_h)

    m3_h = np.zeros((128, 5, 4), dtype=np.float32)
    for kb in range(5):
        kw = KDIMS[kb]
        m3_h[:kw, kb, :] = m3wp[kb * 128 : kb * 128 + kw, :]
    m3_h[112, 4, :] = m3bp
    m3_h = to_bf16(m3_h.reshape(128, 5 * 4))

    ffw_w = np.asarray(ffw_w, dtype=np.float32)
    ffw9_h = np.zeros((9, 13), dtype=np.float32)
    ffw9_h[0:8, 0:12] = ffw_w[0:8]
    ffw9_h[8, 0:12] = np.asarray(ffw_b, dtype=np.float32)
    ffw9_h[8, 12] = 1.0   # ones-row producer (ffin12 row 8 is all-ones)
    ffw9_h = to_bf16(ffw9_h)
    ffw3_h = np.zeros((4, 13), dtype=np.float32)
    ffw3_h[:, 0:12] = ffw_w[8:12]
    ffw3_h = to_bf16(ffw3_h)

    fdw_h = np.zeros((13, 2), dtype=np.float32)
    fdw_h[:12] = np.asarray(fd_w, dtype=np.float32)
    fdw_h[12] = np.asarray(fd_b, dtype=np.float32)
    fdw_h = to_bf16(fdw_h)

    in_maps = []
    for core in range(NCORES):
        sl = slice(core * NPC, (core + 1) * NPC)
        idx_r = (
            idxg[sl].reshape(NT, 128, F).transpose(1, 0, 2).reshape(128, NT * F)
        )
        xv_r = Xv[sl].reshape(NT, 128, F).transpose(1, 0, 2).reshape(128, NT * F)
        in_maps.append(
            dict(
                tab=tab,
                idx=np.ascontiguousarray(idx_r),
                xv=np.ascontiguousarray(xv_r),
                wall=wall_h,
                am=am_h, bm=bm_h, m12=m12_h, m3=m3_h,
                ffw9=ffw9_h, ffw3=ffw3_h, fdw=fdw_h,
            )
        )
    return in_maps


_NC_CACHE = None


def kernel(**inputs):
    global _NC_CACHE, LAST_RESULT
    in_maps = host_pack(**inputs)
    if _NC_CACHE is None:
        _NC_CACHE = build_nc()
    res = run_bass_kernel_spmd(_NC_CACHE, in_maps, list(range(NCORES)))
    LAST_RESULT = res
    out = np.concatenate([res.results[c]["y"] for c in range(NCORES)], axis=0)
    return out


if __name__ == "__main__":
    print("building...")
    nc = build_nc()
    print("built ok")
